# revision 3
# baseline (speedup 1.0000x reference)
"""Trainium2 Bass kernel for supervised contrastive loss over N=8192 rows.

Strategy (8-core SPMD, rows sharded 1024/core):
  - Per column chunk t (128 cols): simT[c, r] = emb_cols_t @ emb_rows.T via PE,
    then exp(sim/T) with the diagonal zeroed by an off-diag mask multiply, then
    S_T[class, row] += onehot_colsT @ exp on PE (bf16).  Classes partition the
    columns, so total_sum = sum_c S_T and positive_sum = S_T[label_r, r]; the
    host gathers S_T per core and finishes the loss (log/mask/mean) in fp64.
  - exp is split across two engines to unblock the PE: pair-groups of chunks
    go through the scalar engine's exact Exp; single-groups are computed on
    the vector engine with a Schraudolph bit-trick (int16(A*sim+B) bitcast to
    bf16 approximates exp(sim/T) with ~2% rms error, mean-zero tuned).
  - The diagonal's chunk position is made core-invariant by rotating each
    core's column-side data (embeddings and one-hots) by its row offset.
"""

import math

import numpy as np
import ml_dtypes

import concourse.tile as tile
from concourse import bacc, mybir
from concourse.bass_utils import run_bass_kernel_spmd

N, D, C = 8192, 128, 100
NCORES = 8
R = N // NCORES  # rows per core
NT = N // 128  # column chunks of 128
RC = R // 128  # row chunks per core (8)
TEMP = 0.07
F32 = mybir.dt.float32
I16 = mybir.dt.int16
BF16 = mybir.dt.bfloat16

# Schraudolph exp for bf16 bit patterns: bf16(int16(A*y + B)) ~= exp(y/T).
# A maps y/T onto base-2 exponent steps of 1/128; B centers the bias
# (127*128) and subtracts 128*log2(E[(1+f)2^-f]) so the piecewise-linear
# mantissa error is mean-zero over uniform fractional parts.
EXP_A = 128.0 / (TEMP * math.log(2.0))
EXP_B = 127.0 * 128.0 - 128.0 * math.log2(1.0406844)

_PROGRAM_CACHE = {}


def _build_program():
    nc = bacc.Bacc("TRN2", target_bir_lowering=False, debug=False, num_devices=NCORES)

    embT_cols = nc.dram_tensor("embT_cols", [D, N], BF16, kind="ExternalInput")
    embT_rows = nc.dram_tensor("embT_rows", [D, R], BF16, kind="ExternalInput")
    ohc = nc.dram_tensor("ohc", [N, C], BF16, kind="ExternalInput")
    offdiag = nc.dram_tensor("offdiag", [128, 128], BF16, kind="ExternalInput")
    S_out = nc.dram_tensor("S_out", [C, R], BF16, kind="ExternalOutput")

    with tile.TileContext(nc) as tc:
        with (
            tc.tile_pool(name="consts", bufs=1) as consts,
            tc.tile_pool(name="spool", bufs=1, space="PSUM") as spool,
            tc.tile_pool(name="simpool", bufs=1, space="PSUM") as simpool,
            tc.tile_pool(name="exppool", bufs=2) as exppool,
            tc.tile_pool(name="dvepool", bufs=2) as dvepool,
            tc.tile_pool(name="fsb", bufs=1) as fsb,
        ):
            # Critical-path loads first, in small pieces, so chunk 0's matmul
            # unblocks as early as possible.  cols chunk 0's first 128 columns
            # go on the otherwise-idle vector queue in parallel with the rows
            # halves on sync.
            rows_sb = consts.tile([D, R], BF16, tag="rows")
            cols_sb = []
            for j in range(8):
                tcol = consts.tile([D, 1024], BF16, tag=f"col{j}", name=f"cols_sb{j}")
                cols_sb.append(tcol)
            ohc_sb = consts.tile([128, NT, C], BF16, tag="ohc")
            ohc_re = ohc[:, :].rearrange("(t p) c -> p t c", p=128)

            nc.scalar.dma_start(cols_sb[0][:, 0:128], embT_cols[:, 0:128])
            nc.sync.dma_start(rows_sb[:, 0:512], embT_rows[:, 0:512])
            nc.sync.dma_start(rows_sb[:, 512:R], embT_rows[:, 512:R])
            nc.sync.dma_start(cols_sb[0][:, 128:1024], embT_cols[:, 128:1024])
            for j in range(1, 8):
                nc.sync.dma_start(cols_sb[j][:], embT_cols[:, j * 1024 : (j + 1) * 1024])
            offd_sb = consts.tile([128, 128], BF16, tag="offd")
            nc.gpsimd.dma_start(ohc_sb[:, 0:2, :], ohc_re[:, 0:2, :])
            nc.gpsimd.dma_start(offd_sb[:], offdiag[:, :])
            nc.gpsimd.dma_start(ohc_sb[:, 2:8, :], ohc_re[:, 2:8, :])
            for j in range(1, 8):
                sl = slice(j * 8, (j + 1) * 8)
                nc.gpsimd.dma_start(ohc_sb[:, sl, :], ohc_re[:, sl, :])

            # S_T[class, row] accumulator over all column chunks. Split into
            # two 512-row tiles: a matmul output must stay within one PSUM bank.
            S_T = [
                spool.tile([C, 512], F32, tag=f"S{q}", name=f"S_T{q}")
                for q in range(2)
            ]

            # Column chunks alternate single/pair groups so one [128, 2048]
            # (4-bank) and one [128, 1024] (2-bank) PSUM tile ping-pong.
            # Pairs are exp'd on the scalar engine; singles on the vector
            # engine (Schraudolph), so neither engine gates the PE.
            groups = []
            t = 0
            while t < NT:
                if len(groups) % 2 == 1 and t + 1 < NT:
                    groups.append((t, t + 1))
                    t += 2
                else:
                    groups.append((t,))
                    t += 1

            exp_of_group = [None] * len(groups)

            def emit_sim_exp(g):
                chunks = groups[g]
                n = len(chunks)
                tag = "simbig" if n == 2 else "simsmall"
                sim_ps = simpool.tile([128, n * R], F32, name=f"sim{g}", tag=tag)
                for i, tt in enumerate(chunks):
                    lhsT = cols_sb[tt // 8][:, (tt % 8) * 128 : (tt % 8 + 1) * 128]
                    for h in range(2):
                        osl = slice(i * R + h * 512, i * R + (h + 1) * 512)
                        rsl = slice(h * 512, (h + 1) * 512)
                        nc.tensor.matmul(
                            sim_ps[:, osl], lhsT, rows_sb[:, rsl], start=True, stop=True
                        )
                if n == 2:
                    exp_sb = exppool.tile([128, n * R], BF16, name=f"exp{g}", tag="expbig")
                    nc.scalar.activation(
                        exp_sb[:], sim_ps[:], mybir.ActivationFunctionType.Exp,
                        scale=float(1.0 / TEMP),
                    )
                else:
                    exp_sb = dvepool.tile([128, R], BF16, name=f"exp{g}", tag="dve")
                    nc.vector.tensor_scalar(
                        exp_sb[:].bitcast(I16), sim_ps[:],
                        float(EXP_A), float(EXP_B),
                        mybir.AluOpType.mult, mybir.AluOpType.add,
                    )
                for i, tt in enumerate(chunks):
                    if tt < RC:
                        # Chunk tt's columns are rows tt*128..tt*128+127 of this
                        # core: the diagonal is the main diagonal of the block.
                        blk = slice(i * R + tt * 128, i * R + (tt + 1) * 128)
                        nc.vector.tensor_mul(
                            exp_sb[:, blk], exp_sb[:, blk], offd_sb[:]
                        )
                exp_of_group[g] = exp_sb

            emit_sim_exp(0)
            emit_sim_exp(1)
            for g, chunks in enumerate(groups):
                if g + 2 < len(groups):
                    emit_sim_exp(g + 2)
                for i, tt in enumerate(chunks):
                    for q in range(2):
                        nc.tensor.matmul(
                            S_T[q][:],
                            ohc_sb[:, tt, :],
                            exp_of_group[g][:, i * R + q * 512 : i * R + (q + 1) * 512],
                            start=(tt == 0),
                            stop=(tt == NT - 1),
                        )

            # Tail: ship S_T to the host (cast to bf16 in SBUF first — DMA
            # cannot read PSUM).  Interleave the two halves so DMA of half 0
            # overlaps the cast of half 1.
            S_sb = fsb.tile([C, R], BF16, tag="S_sb")
            for q in range(2):
                sl = slice(q * 512, (q + 1) * 512)
                nc.vector.tensor_copy(S_sb[:, sl], S_T[q][:])
                nc.sync.dma_start(S_out[:, sl], S_sb[:, sl])

    nc.compile()
    return nc


def _get_program():
    if "p" not in _PROGRAM_CACHE:
        _PROGRAM_CACHE["p"] = _build_program()
    return _PROGRAM_CACHE["p"]


def _prepare_in_maps(embeddings, labels):
    emb = np.asarray(embeddings, dtype=np.float32)
    lab = np.asarray(labels).astype(np.int64)
    embT = np.ascontiguousarray(emb.T).astype(ml_dtypes.bfloat16)  # [D, N]
    classes = np.arange(C, dtype=np.int64)
    onehot = lab[:, None] == classes[None, :]  # [N, C] bool
    oh_bf16 = onehot.astype(ml_dtypes.bfloat16)
    offd = (1.0 - np.eye(128, dtype=np.float32)).astype(ml_dtypes.bfloat16)

    in_maps = []
    for i in range(NCORES):
        r0 = i * R
        in_maps.append(
            {
                "embT_cols": np.ascontiguousarray(np.roll(embT, -r0, axis=1)),
                "embT_rows": np.ascontiguousarray(embT[:, r0 : r0 + R]),
                "ohc": np.ascontiguousarray(np.roll(oh_bf16, -r0, axis=0)),
                "offdiag": offd,
            }
        )
    return in_maps, lab


def run(embeddings, labels, trace=False, trace_cores=None):
    """Returns (mean_loss, BassKernelResults)."""
    nc = _get_program()
    in_maps, lab = _prepare_in_maps(embeddings, labels)
    kwargs = {}
    if trace:
        kwargs["trace"] = True
        if trace_cores is not None:
            kwargs["trace_cores"] = trace_cores
    res = run_bass_kernel_spmd(nc, in_maps, core_ids=list(range(NCORES)), **kwargs)

    # Host tail: S[class, row] per core -> loss (fp64).
    counts = np.bincount(lab, minlength=C)
    valid = (counts[lab] - 1) > 0  # [N]
    loss_sum = 0.0
    for i in range(NCORES):
        S = res.results[i]["S_out"].astype(np.float64)  # [C, R]
        r0 = i * R
        lab_i = lab[r0 : r0 + R]
        tot = S.sum(axis=0)
        pos = S[lab_i, np.arange(R)]
        loss = -np.log(pos / (tot + 1e-8) + 1e-8)
        loss_sum += loss[valid[r0 : r0 + R]].sum()
    cnt = int(valid.sum())
    mean = loss_sum / cnt if cnt > 0 else 0.0
    return np.asarray(mean, dtype=np.float32), res


def kernel(embeddings, labels):
    return run(embeddings, labels)[0]
